# revision 19
# baseline (speedup 1.0000x reference)
"""Trainium2 Bass kernel for nn_CoreAmplifierLM (minGRU LM, 4 blocks).

Strategy (8 NeuronCores, SPMD):
  - Sequence-parallel blocks: core k owns tokens [k*512, (k+1)*512) plus a
    redundant W=128-token warmup prefix (recurrence decay prod(a) <= 1.5e-3
    over 128 tokens makes the h=0 warm start exact to ~1e-4 in the logits).
    Core 0's warmup wraps the sequence end; a b-mask zeroes its scan input.
  - fp16 residual stream: the embedding table is host-cast to fp16, x^T
    lives as [P, KD, TW] fp16 (D on partitions). Each block processes one
    128-token warmup chunk and ONE merged 512-token main chunk (a [128,512]
    fp32 PSUM tile is exactly one bank and 512 is the max moving-operand
    free size), minimizing per-op overhead and engine-queue boundaries.
    g(h) uses the exact identity relu(h)+sigmoid(min(h,0)) = max(h+0.5,
    sigmoid(h)). Work is spread over scalar (sigmoids, sqrt), vector
    (norms, g(), b, scans) and gpsimd (1-z, residual adds, carries).
    The next block's norms are emitted inside the current block so chunk
    matmuls never wait on the norm chain.
  - Readout is vocab-sharded (tensor parallel): each core holds an fp16
    [512, 4000] readout slice in SBUF (preloaded at t=0) and computes
    logits[:, vslice] for all 4096 tokens. The final-normed x is exchanged
    with TWO pipelined 256-token AllGathers (the first fires mid-block-4;
    the last block keeps split 256-token chunks for this), and the readout
    consumes token tiles in piece-arrival order so the exchange mostly
    hides under matmul work. Logits are assembled per 128-token tile into
    a [128, 4000] fp16 staging tile and written with one contiguous 1 MB
    DMA, widened on host.
"""
import numpy as np
from contextlib import ExitStack

import concourse.bass as bass
import concourse.mybir as mybir
import concourse.tile as tile
from concourse import bacc
from concourse.bass_utils import run_bass_kernel_spmd

P = 128
D = 512
V = 32000
SEQ = 4096
NB = 4
NCORES = 8
CHUNK = SEQ // NCORES          # 512 own tokens per core
W = 128                        # warmup tokens
TW = CHUNK + W                 # 640-token window per core
PIECE = 256                    # AllGather piece (tokens)
NPIECE = CHUNK // PIECE        # 2 pieces per core
KD = D // P                    # 4 contraction chunks
MH = 2 * D // P                # 8 output-channel chunks of hg
NG = KD                        # 4 channel groups (hidden dim)
VSH = V // NCORES              # 4000 vocab per core
VB = 500                       # vocab block for readout (one PSUM bank)
NVB = VSH // VB                # 8
TM = SEQ // P                  # 32 token m-chunks in readout
EPS = 1e-6

# per-block token chunks (start, width): one merged 512-token main chunk,
# except the last block which splits it so the first AllGather fires early
CHUNKS_MAIN = [(0, W), (W, 512)]
CHUNKS_LAST = [(0, W), (W, 256), (W + 256, 256)]

F32 = mybir.dt.float32
F16 = mybir.dt.float16
I32 = mybir.dt.int32
AF = mybir.ActivationFunctionType
OP = mybir.AluOpType

_CACHE = {}


def _build(reps=1):
    nc = bacc.Bacc("TRN2", target_bir_lowering=False, debug=False,
                   enable_asserts=True, num_devices=NCORES)

    emb = nc.dram_tensor("emb", [V, D], F16, kind="ExternalInput").ap()
    idx = nc.dram_tensor("idx", [P, TW // P], I32, kind="ExternalInput").ap()
    wq = nc.dram_tensor("wq", [NB, D, 2 * D], F16, kind="ExternalInput").ap()
    wro = nc.dram_tensor("wro", [D, VSH], F16, kind="ExternalInput").ap()
    ones_in = nc.dram_tensor("ones_in", [P, P], F16, kind="ExternalInput").ap()
    ident_in = nc.dram_tensor("ident_in", [P, P], F16,
                              kind="ExternalInput").ap()
    eps_in = nc.dram_tensor("eps_in", [P, 1], F32, kind="ExternalInput").ap()
    bmask = nc.dram_tensor("bmask", [P, W], F16, kind="ExternalInput").ap()
    out = nc.dram_tensor("out", [SEQ, VSH], F16, kind="ExternalOutput").ap()
    cc_in = [nc.dram_tensor(f"cc_in{p}", [P, KD, PIECE], F16,
                            kind="Internal").ap() for p in range(NPIECE)]
    cc_out = [nc.dram_tensor(f"cc_out{p}", [NCORES, P, KD, PIECE], F16,
                             kind="Internal", addr_space="Shared").ap()
              for p in range(NPIECE)]

    with tile.TileContext(nc) as tc, ExitStack() as ctx:
        cpool = ctx.enter_context(tc.tile_pool(name="const", bufs=1))
        xpool = ctx.enter_context(tc.tile_pool(name="xT", bufs=1))
        gpool = ctx.enter_context(tc.tile_pool(name="gather", bufs=3))
        wpool = ctx.enter_context(tc.tile_pool(name="w", bufs=2))
        tpool = ctx.enter_context(tc.tile_pool(name="normtmp", bufs=1))
        epool = ctx.enter_context(tc.tile_pool(name="elem", bufs=3))
        hpool = ctx.enter_context(tc.tile_pool(name="h", bufs=3))
        opool = ctx.enter_context(tc.tile_pool(name="obuf", bufs=3))

        # tiny front-loaded constants (sync queue) so the gather can start
        idx_sb = cpool.tile([P, TW // P], I32)
        nc.sync.dma_start(idx_sb[:], idx)
        mask_sb = cpool.tile([P, W], F16)
        nc.sync.dma_start(mask_sb[:], bmask)
        ones_h = cpool.tile([P, P], F16)
        nc.sync.dma_start(ones_h[:], ones_in)
        ident = cpool.tile([P, P], F16)
        nc.sync.dma_start(ident[:], ident_in)
        carry = cpool.tile([P, NG], F16)
        eps_sb = cpool.tile([P, 1], F32)
        nc.sync.dma_start(eps_sb[:], eps_in)
        # big preload: the fp16 readout slice rides the sync queue behind the
        # tiny constants (it is only needed at readout time, ~150us in)
        wro_sb = cpool.tile([P, KD, VSH], F16)
        nc.sync.dma_start(wro_sb[:], wro.rearrange("(kd p) v -> p kd v", p=P))

        args = (nc, tc, ctx, cpool, xpool, gpool, wpool, tpool, epool, hpool,
                opool, ident, ones_h, mask_sb, idx_sb, carry,
                eps_sb, wro_sb, emb, wq, out, cc_in, cc_out)
        if reps == 1:
            _body(*args)
        else:
            with tc.For_i(0, reps, 1):
                _body(*args)

    nc.compile()
    return nc


def _body(nc, tc, ctx, cpool, xpool, gpool, wpool, tpool, epool, hpool,
          opool, ident, ones_h, mask_sb, idx_sb, carry,
          eps_sb, wro_sb, emb, wq, out, cc_in, cc_out):
    psbox = [None]
    with tc.tile_pool(name="psb", bufs=1, space="PSUM") as pspool:
        psbox[0] = (pspool, "ps", 7)
        # ---- phase A: gather embedding rows, transpose to xT [P, KD, TW] ----
        xT = xpool.tile([P, KD, TW], F16)

        def gather_group(g):
            xr = gpool.tile([P, D], F16, tag="xr")
            nc.gpsimd.indirect_dma_start(
                out=xr[:], out_offset=None, in_=emb,
                in_offset=bass.IndirectOffsetOnAxis(ap=idx_sb[:, g:g + 1], axis=0))
            for d in range(KD):
                ps_t = psbox[0][0].tile([P, 2 * 512], F16, tag="pst", bufs=1,
                                        name="ps_t")
                nc.tensor.transpose(ps_t[:, :P], xr[:, d * P:(d + 1) * P],
                                    ident[:])
                nc.vector.tensor_copy(xT[:, d, g * P:(g + 1) * P], ps_t[:, :P])

        def norm_begin(clo, width, tag):
            """Incremental rms-norm: on_group(g) squares + accumulates group
            g's partition-sum right after that group's residual add, so the
            norm finishes ~one op after the chunk's last add."""
            csl = slice(clo, clo + width)
            pool, ptag, pbufs = psbox[0]
            ps_n = pool.tile([P, 512], F32, tag=ptag, bufs=pbufs,
                             name=f"psn_{tag}")

            def on_group(g):
                x2g = tpool.tile([P, width], F16, tag=f"x2g{width}", bufs=2,
                                 name=f"x2g_{tag}_{g}")
                nc.vector.tensor_tensor(x2g[:], xT[:, g, csl], xT[:, g, csl],
                                        op=OP.mult)
                nc.tensor.matmul(ps_n[:, :width], lhsT=ones_h[:], rhs=x2g[:],
                                 start=(g == 0), stop=(g == NG - 1))

            def finish(xn_tag):
                # rstd = 1/sqrt(mean + eps), one table op (input is >= 0)
                rstd_h = tpool.tile([P, width], F16, tag=f"rstdh{width}",
                                    name=f"rs_{tag}")
                nc.scalar.activation(rstd_h[:], ps_n[:, :width],
                                     AF.Abs_reciprocal_sqrt,
                                     scale=1.0 / D, bias=eps_sb[:, :1])
                xn = tpool.tile([P, KD, width], F16, tag=xn_tag,
                                name=f"xn_{tag}")
                nc.vector.tensor_tensor(
                    xn[:], xT[:, :, csl],
                    rstd_h[:, None, :].to_broadcast([P, KD, width]),
                    op=OP.mult)
                return xn

            return on_group, finish

        def rms_xn(clo, width, xn_tag="xn"):
            og, fin = norm_begin(clo, width, xn_tag)
            for g in range(NG):
                og(g)
            return fin(xn_tag)

        # the first norm only needs gather group 0; hoisting it between the
        # gathers lets block 0 start earlier
        gather_group(0)
        xn00 = rms_xn(0, W, xn_tag="xn0")
        for g in range(1, TW // P):
            gather_group(g)

        # ---- phase B: minGRU blocks ----
        next_xns = None
        for i in range(NB):
            chunks = CHUNKS_LAST if i == NB - 1 else CHUNKS_MAIN
            nchunks = CHUNKS_LAST if i == NB - 2 else CHUNKS_MAIN
            w_sb = wpool.tile([P, KD, MH, P], F16, tag="w")
            nc.scalar.dma_start(
                w_sb[:], wq[i].rearrange("(kd p) (mh j) -> p kd mh j", p=P, j=P))
            if i == 0:
                xns = [xn00] + [rms_xn(clo, cw, xn_tag=f"b0xn{c}")
                                for c, (clo, cw) in enumerate(chunks) if c > 0]
            else:
                xns = next_xns
            next_xns = [None] * len(nchunks)
            for c, (clo, cw) in enumerate(chunks):
                csl = slice(clo, clo + cw)
                warm = c == 0
                xn = xns[c]
                # pending norms over this chunk's tokens, finished right
                # after the last group's residual add lands
                pend = []
                if i < NB - 1:
                    if warm:
                        pend = [(0, norm_begin(0, W, f"n{i}c0"))]
                    else:
                        pend = [(j, norm_begin(nlo, nw, f"n{i}c{j}"))
                                for j, (nlo, nw) in enumerate(nchunks)
                                if j > 0]
                elif c >= 1:
                    pend = [(c, norm_begin(clo, cw, f"pc{c}"))]
                for g in range(NG):
                    ps_h = pspool.tile([P, 512], F32, tag="ps", bufs=7)
                    ps_g = pspool.tile([P, 512], F32, tag="ps", bufs=7)
                    for kd in range(KD):
                        nc.tensor.matmul(ps_h[:, :cw], lhsT=w_sb[:, kd, g, :],
                                         rhs=xn[:, kd, :],
                                         start=(kd == 0), stop=(kd == KD - 1))
                    for kd in range(KD):
                        nc.tensor.matmul(ps_g[:, :cw], lhsT=w_sb[:, kd, g + NG, :],
                                         rhs=xn[:, kd, :],
                                         start=(kd == 0), stop=(kd == KD - 1))
                    z = epool.tile([P, 512], F16, tag="z")
                    nc.scalar.activation(z[:, :cw], ps_g[:, :cw], AF.Sigmoid)
                    # a = 1 - z = sigmoid(-gate), off the scalar engine
                    a = epool.tile([P, 512], F16, tag="a")
                    nc.gpsimd.tensor_scalar(a[:, :cw], z[:, :cw], -1.0, 1.0,
                                            op0=OP.mult, op1=OP.add)
                    sh = epool.tile([P, 512], F16, tag="sh")
                    nc.scalar.activation(sh[:, :cw], ps_h[:, :cw], AF.Sigmoid)
                    # g(h) = relu(h) + sigmoid(min(h, 0)) = max(h + 0.5,
                    # sigmoid(h)) exactly (sigmoid is below h+0.5 iff h > 0)
                    gg = epool.tile([P, 512], F16, tag="gg")
                    nc.vector.scalar_tensor_tensor(
                        out=gg[:, :cw], in0=ps_h[:, :cw], scalar=0.5,
                        in1=sh[:, :cw], op0=OP.add, op1=OP.max)
                    if warm:
                        # zero warmup b on core 0 (mask = 0 there, 1 elsewhere)
                        nc.vector.tensor_tensor(z[:, :cw], z[:, :cw],
                                                mask_sb[:], op=OP.mult)
                    b = epool.tile([P, 512], F16, tag="b")
                    nc.vector.tensor_tensor(b[:, :cw], z[:, :cw], gg[:, :cw],
                                            op=OP.mult)
                    h = hpool.tile([P, 512], F16, tag="h")
                    init = 0.0 if warm else carry[:, g:g + 1]
                    nc.vector.tensor_tensor_scan(
                        out=h[:, :cw], data0=a[:, :cw], data1=b[:, :cw],
                        initial=init, op0=OP.mult, op1=OP.add)
                    if c < len(chunks) - 1:
                        nc.gpsimd.tensor_copy(carry[:, g:g + 1], h[:, cw - 1:cw])
                    nc.gpsimd.tensor_tensor(xT[:, g, csl], xT[:, g, csl],
                                            h[:, :cw], op=OP.add)
                    for _, (og, _fin) in pend:
                        og(g)
                if i < NB - 1:
                    for j, (_og, fin) in pend:
                        next_xns[j] = fin(f"n{i}x{j}")
                elif c >= 1:
                    # own piece is final: norm + AllGather it immediately
                    # (piece 0 fires while the last chunk still computes)
                    _, (_og, fin) = pend[0]
                    xf = fin(f"xf{c - 1}")
                    nc.sync.dma_start(cc_in[c - 1], xf[:])
                    nc.gpsimd.collective_compute(
                        kind="AllGather", op=OP.bypass,
                        replica_groups=[list(range(NCORES))],
                        ins=[cc_in[c - 1]], outs=[cc_out[c - 1]])

        # ---- phase C: regather ----
        xg_all = xpool.tile([P, NCORES, NPIECE, KD, PIECE], F16)
        for p in range(NPIECE):
            for c in range(NCORES):
                nc.gpsimd.dma_start(xg_all[:, c, p], cc_out[p][c])

    # ---- readout (own PSUM pool: 8 single-bank tiles) ----
    # token tiles in piece-arrival order: all cores' piece 0, then piece 1
    tiles = [(c, p, hf) for p in range(NPIECE) for c in range(NCORES)
             for hf in range(2)]
    with tc.tile_pool(name="psro", bufs=8, space="PSUM") as rpool:
        for ti, (c, p, hf) in enumerate(tiles):
            tm = c * 4 + p * 2 + hf
            ob = opool.tile([P, VSH], F16, tag="ob")
            for vbg in range(2):
                pss = [rpool.tile([P, VB], F32, tag="pso", name=f"pso{j}")
                       for j in range(4)]
                for kd in range(KD):
                    for j in range(4):
                        vb = vbg * 4 + j
                        nc.tensor.matmul(
                            pss[j][:],
                            lhsT=xg_all[:, c, p, kd, hf * P:(hf + 1) * P],
                            rhs=wro_sb[:, kd, vb * VB:(vb + 1) * VB],
                            start=(kd == 0), stop=(kd == KD - 1))
                for j in range(4):
                    vb = vbg * 4 + j
                    osl = ob[:, vb * VB:(vb + 1) * VB]
                    if j % 2 == 0:
                        nc.scalar.activation(osl, pss[j][:], AF.Copy)
                    else:
                        nc.vector.tensor_copy(osl, pss[j][:])
            dma_eng = nc.sync if ti % 2 == 0 else nc.scalar
            dma_eng.dma_start(out[tm * P:(tm + 1) * P, :], ob[:])


def _get_nc(reps=1):
    key = ("nc", reps)
    if key not in _CACHE:
        _CACHE[key] = _build(reps)
    return _CACHE[key]


def _make_in_maps(input_ids, token_embed, w_hg, norm_scales, final_scale,
                  readout_weight):
    ids = np.asarray(input_ids).reshape(-1).astype(np.int64)
    emb = np.ascontiguousarray(np.asarray(token_embed, np.float32)
                               ).astype(np.float16)
    wq = np.ascontiguousarray(
        np.asarray(norm_scales, np.float32)[:, :, None]
        * np.asarray(w_hg, np.float32)).astype(np.float16)
    wro_full = (np.asarray(final_scale, np.float32)[:, None]
                * np.asarray(readout_weight, np.float32))
    ones = np.ones((P, P), np.float16)
    identm = np.eye(P, dtype=np.float16)
    epsc = np.full((P, 1), EPS, np.float32)
    in_maps = []
    for core in range(NCORES):
        start = core * CHUNK
        widx = (np.arange(start - W, start + CHUNK) % SEQ).astype(np.int64)
        # partition-major layout: idx[p, g] = window token g*128 + p
        idxm = np.ascontiguousarray(
            ids[widx].astype(np.int32).reshape(TW // P, P).T)
        mask = np.ones((P, W), np.float16)
        if core == 0:
            mask[:] = 0.0
        wro = np.ascontiguousarray(
            wro_full[:, core * VSH:(core + 1) * VSH]).astype(np.float16)
        in_maps.append(dict(emb=emb, idx=idxm, wq=wq, wro=wro, ones_in=ones,
                            ident_in=identm, eps_in=epsc, bmask=mask))
    return in_maps


def kernel(input_ids, token_embed, w_hg, norm_scales, final_scale,
           readout_weight):
    nc = _get_nc()
    in_maps = _make_in_maps(input_ids, token_embed, w_hg, norm_scales,
                            final_scale, readout_weight)
    res = run_bass_kernel_spmd(nc, in_maps, core_ids=list(range(NCORES)))
    logits = np.concatenate(
        [np.asarray(res.results[c]["out"]).astype(np.float32)
         for c in range(NCORES)], axis=1)
    return logits.reshape(1, SEQ, V)


# revision 20
# speedup vs baseline: 1.0190x; 1.0190x over previous
"""Trainium2 Bass kernel for nn_CoreAmplifierLM (minGRU LM, 4 blocks).

Strategy (8 NeuronCores, SPMD):
  - Sequence-parallel blocks: core k owns tokens [k*512, (k+1)*512) plus a
    redundant W=128-token warmup prefix (recurrence decay prod(a) <= 1.5e-3
    over 128 tokens makes the h=0 warm start exact to ~1e-4 in the logits).
    Core 0's warmup wraps the sequence end; a b-mask zeroes its scan input.
  - fp16 residual stream: the embedding table is host-cast to fp16, x^T
    lives as [P, KD, TW] fp16 (D on partitions). Each block processes one
    128-token warmup chunk and ONE merged 512-token main chunk (a [128,512]
    fp32 PSUM tile is exactly one bank and 512 is the max moving-operand
    free size), minimizing per-op overhead and engine-queue boundaries.
    g(h) uses the exact identity relu(h)+sigmoid(min(h,0)) = max(h+0.5,
    sigmoid(h)). Work is spread over scalar (sigmoids, sqrt), vector
    (norms, g(), b, scans) and gpsimd (1-z, residual adds, carries).
    The next block's norms are emitted inside the current block so chunk
    matmuls never wait on the norm chain.
  - Readout is vocab-sharded (tensor parallel): each core holds an fp16
    [512, 4000] readout slice in SBUF (preloaded at t=0) and computes
    logits[:, vslice] for all 4096 tokens. The final-normed x is exchanged
    with TWO pipelined 256-token AllGathers (the first fires mid-block-4;
    the last block keeps split 256-token chunks for this), and the readout
    consumes token tiles in piece-arrival order so the exchange mostly
    hides under matmul work. Logits are assembled per 128-token tile into
    a [128, 4000] fp16 staging tile and written with one contiguous 1 MB
    DMA, widened on host.
"""
import numpy as np
from contextlib import ExitStack

import concourse.bass as bass
import concourse.mybir as mybir
import concourse.tile as tile
from concourse import bacc
from concourse.bass_utils import run_bass_kernel_spmd

P = 128
D = 512
V = 32000
SEQ = 4096
NB = 4
NCORES = 8
CHUNK = SEQ // NCORES          # 512 own tokens per core
W = 128                        # warmup window (gather-layout granularity)
WS, WW = 64, 64                # effective warm scan: tokens [64, 128)
TW = CHUNK + W                 # 640-token window per core
PIECE = 256                    # AllGather piece (tokens)
NPIECE = CHUNK // PIECE        # 2 pieces per core
KD = D // P                    # 4 contraction chunks
MH = 2 * D // P                # 8 output-channel chunks of hg
NG = KD                        # 4 channel groups (hidden dim)
VSH = V // NCORES              # 4000 vocab per core
VB = 500                       # vocab block for readout (one PSUM bank)
NVB = VSH // VB                # 8
TM = SEQ // P                  # 32 token m-chunks in readout
EPS = 1e-6

# per-block token chunks (start, width): one merged 512-token main chunk,
# except the last block which splits it so the first AllGather fires early
CHUNKS_MAIN = [(WS, WW), (W, 512)]
CHUNKS_LAST = [(WS, WW), (W, 256), (W + 256, 256)]

F32 = mybir.dt.float32
F16 = mybir.dt.float16
I32 = mybir.dt.int32
AF = mybir.ActivationFunctionType
OP = mybir.AluOpType

_CACHE = {}


def _build(reps=1):
    nc = bacc.Bacc("TRN2", target_bir_lowering=False, debug=False,
                   enable_asserts=True, num_devices=NCORES)

    emb = nc.dram_tensor("emb", [V, D], F16, kind="ExternalInput").ap()
    idx = nc.dram_tensor("idx", [P, TW // P], I32, kind="ExternalInput").ap()
    wq = nc.dram_tensor("wq", [NB, D, 2 * D], F16, kind="ExternalInput").ap()
    wro = nc.dram_tensor("wro", [D, VSH], F16, kind="ExternalInput").ap()
    ones_in = nc.dram_tensor("ones_in", [P, P], F16, kind="ExternalInput").ap()
    ident_in = nc.dram_tensor("ident_in", [P, P], F16,
                              kind="ExternalInput").ap()
    eps_in = nc.dram_tensor("eps_in", [P, 1], F32, kind="ExternalInput").ap()
    bmask = nc.dram_tensor("bmask", [P, WW], F16, kind="ExternalInput").ap()
    out = nc.dram_tensor("out", [SEQ, VSH], F16, kind="ExternalOutput").ap()
    cc_in = [nc.dram_tensor(f"cc_in{p}", [P, KD, PIECE], F16,
                            kind="Internal").ap() for p in range(NPIECE)]
    cc_out = [nc.dram_tensor(f"cc_out{p}", [NCORES, P, KD, PIECE], F16,
                             kind="Internal", addr_space="Shared").ap()
              for p in range(NPIECE)]

    with tile.TileContext(nc) as tc, ExitStack() as ctx:
        cpool = ctx.enter_context(tc.tile_pool(name="const", bufs=1))
        xpool = ctx.enter_context(tc.tile_pool(name="xT", bufs=1))
        gpool = ctx.enter_context(tc.tile_pool(name="gather", bufs=3))
        wpool = ctx.enter_context(tc.tile_pool(name="w", bufs=2))
        tpool = ctx.enter_context(tc.tile_pool(name="normtmp", bufs=1))
        epool = ctx.enter_context(tc.tile_pool(name="elem", bufs=3))
        hpool = ctx.enter_context(tc.tile_pool(name="h", bufs=3))
        opool = ctx.enter_context(tc.tile_pool(name="obuf", bufs=3))

        # tiny front-loaded constants (sync queue) so the gather can start
        idx_sb = cpool.tile([P, TW // P], I32)
        nc.sync.dma_start(idx_sb[:], idx)
        mask_sb = cpool.tile([P, WW], F16)
        nc.sync.dma_start(mask_sb[:], bmask)
        ones_h = cpool.tile([P, P], F16)
        nc.sync.dma_start(ones_h[:], ones_in)
        ident = cpool.tile([P, P], F16)
        nc.sync.dma_start(ident[:], ident_in)
        carry = cpool.tile([P, NG], F16)
        eps_sb = cpool.tile([P, 1], F32)
        nc.sync.dma_start(eps_sb[:], eps_in)
        # big preload: the fp16 readout slice rides the sync queue behind the
        # tiny constants (it is only needed at readout time, ~150us in)
        wro_sb = cpool.tile([P, KD, VSH], F16)
        nc.sync.dma_start(wro_sb[:], wro.rearrange("(kd p) v -> p kd v", p=P))

        args = (nc, tc, ctx, cpool, xpool, gpool, wpool, tpool, epool, hpool,
                opool, ident, ones_h, mask_sb, idx_sb, carry,
                eps_sb, wro_sb, emb, wq, out, cc_in, cc_out)
        if reps == 1:
            _body(*args)
        else:
            with tc.For_i(0, reps, 1):
                _body(*args)

    nc.compile()
    return nc


def _body(nc, tc, ctx, cpool, xpool, gpool, wpool, tpool, epool, hpool,
          opool, ident, ones_h, mask_sb, idx_sb, carry,
          eps_sb, wro_sb, emb, wq, out, cc_in, cc_out):
    psbox = [None]
    with tc.tile_pool(name="psb", bufs=1, space="PSUM") as pspool:
        psbox[0] = (pspool, "ps", 7)
        # ---- phase A: gather embedding rows, transpose to xT [P, KD, TW] ----
        xT = xpool.tile([P, KD, TW], F16)

        def gather_group(g):
            xr = gpool.tile([P, D], F16, tag="xr")
            nc.gpsimd.indirect_dma_start(
                out=xr[:], out_offset=None, in_=emb,
                in_offset=bass.IndirectOffsetOnAxis(ap=idx_sb[:, g:g + 1], axis=0))
            for d in range(KD):
                ps_t = psbox[0][0].tile([P, 2 * 512], F16, tag="pst", bufs=1,
                                        name="ps_t")
                nc.tensor.transpose(ps_t[:, :P], xr[:, d * P:(d + 1) * P],
                                    ident[:])
                nc.vector.tensor_copy(xT[:, d, g * P:(g + 1) * P], ps_t[:, :P])

        def norm_begin(clo, width, tag):
            """Incremental rms-norm: on_group(g) squares + accumulates group
            g's partition-sum right after that group's residual add, so the
            norm finishes ~one op after the chunk's last add."""
            csl = slice(clo, clo + width)
            pool, ptag, pbufs = psbox[0]
            ps_n = pool.tile([P, 512], F32, tag=ptag, bufs=pbufs,
                             name=f"psn_{tag}")

            def on_group(g):
                x2g = tpool.tile([P, width], F16, tag=f"x2g{width}", bufs=2,
                                 name=f"x2g_{tag}_{g}")
                nc.vector.tensor_tensor(x2g[:], xT[:, g, csl], xT[:, g, csl],
                                        op=OP.mult)
                nc.tensor.matmul(ps_n[:, :width], lhsT=ones_h[:], rhs=x2g[:],
                                 start=(g == 0), stop=(g == NG - 1))

            def finish(xn_tag):
                # rstd = 1/sqrt(mean + eps), one table op (input is >= 0)
                rstd_h = tpool.tile([P, width], F16, tag=f"rstdh{width}",
                                    name=f"rs_{tag}")
                nc.scalar.activation(rstd_h[:], ps_n[:, :width],
                                     AF.Abs_reciprocal_sqrt,
                                     scale=1.0 / D, bias=eps_sb[:, :1])
                xn = tpool.tile([P, KD, width], F16, tag=xn_tag,
                                name=f"xn_{tag}")
                nc.vector.tensor_tensor(
                    xn[:], xT[:, :, csl],
                    rstd_h[:, None, :].to_broadcast([P, KD, width]),
                    op=OP.mult)
                return xn

            return on_group, finish

        def rms_xn(clo, width, xn_tag="xn"):
            og, fin = norm_begin(clo, width, xn_tag)
            for g in range(NG):
                og(g)
            return fin(xn_tag)

        # the first norm only needs gather group 0; hoisting it between the
        # gathers lets block 0 start earlier
        gather_group(0)
        xn00 = rms_xn(WS, WW, xn_tag="xn0")
        for g in range(1, TW // P):
            gather_group(g)

        # ---- phase B: minGRU blocks ----
        next_xns = None
        for i in range(NB):
            chunks = CHUNKS_LAST if i == NB - 1 else CHUNKS_MAIN
            nchunks = CHUNKS_LAST if i == NB - 2 else CHUNKS_MAIN
            w_sb = wpool.tile([P, KD, MH, P], F16, tag="w")
            nc.scalar.dma_start(
                w_sb[:], wq[i].rearrange("(kd p) (mh j) -> p kd mh j", p=P, j=P))
            if i == 0:
                xns = [xn00] + [rms_xn(clo, cw, xn_tag=f"b0xn{c}")
                                for c, (clo, cw) in enumerate(chunks) if c > 0]
            else:
                xns = next_xns
            next_xns = [None] * len(nchunks)
            for c, (clo, cw) in enumerate(chunks):
                csl = slice(clo, clo + cw)
                warm = c == 0
                xn = xns[c]
                # pending norms over this chunk's tokens, finished right
                # after the last group's residual add lands
                pend = []
                if i < NB - 1:
                    if warm:
                        pend = [(0, norm_begin(WS, WW, f"n{i}c0"))]
                    else:
                        pend = [(j, norm_begin(nlo, nw, f"n{i}c{j}"))
                                for j, (nlo, nw) in enumerate(nchunks)
                                if j > 0]
                elif c >= 1:
                    pend = [(c, norm_begin(clo, cw, f"pc{c}"))]
                for g in range(NG):
                    ps_h = pspool.tile([P, 512], F32, tag="ps", bufs=7)
                    ps_g = pspool.tile([P, 512], F32, tag="ps", bufs=7)
                    for kd in range(KD):
                        nc.tensor.matmul(ps_h[:, :cw], lhsT=w_sb[:, kd, g, :],
                                         rhs=xn[:, kd, :],
                                         start=(kd == 0), stop=(kd == KD - 1))
                    for kd in range(KD):
                        nc.tensor.matmul(ps_g[:, :cw], lhsT=w_sb[:, kd, g + NG, :],
                                         rhs=xn[:, kd, :],
                                         start=(kd == 0), stop=(kd == KD - 1))
                    z = epool.tile([P, 512], F16, tag="z")
                    nc.scalar.activation(z[:, :cw], ps_g[:, :cw], AF.Sigmoid)
                    # a = 1 - z = sigmoid(-gate), off the scalar engine
                    a = epool.tile([P, 512], F16, tag="a")
                    nc.gpsimd.tensor_scalar(a[:, :cw], z[:, :cw], -1.0, 1.0,
                                            op0=OP.mult, op1=OP.add)
                    sh = epool.tile([P, 512], F16, tag="sh")
                    nc.scalar.activation(sh[:, :cw], ps_h[:, :cw], AF.Sigmoid)
                    # g(h) = relu(h) + sigmoid(min(h, 0)) = max(h + 0.5,
                    # sigmoid(h)) exactly (sigmoid is below h+0.5 iff h > 0)
                    gg = epool.tile([P, 512], F16, tag="gg")
                    nc.vector.scalar_tensor_tensor(
                        out=gg[:, :cw], in0=ps_h[:, :cw], scalar=0.5,
                        in1=sh[:, :cw], op0=OP.add, op1=OP.max)
                    if warm:
                        # zero warmup b on core 0 (mask = 0 there, 1 elsewhere)
                        nc.vector.tensor_tensor(z[:, :cw], z[:, :cw],
                                                mask_sb[:], op=OP.mult)
                    b = epool.tile([P, 512], F16, tag="b")
                    nc.vector.tensor_tensor(b[:, :cw], z[:, :cw], gg[:, :cw],
                                            op=OP.mult)
                    h = hpool.tile([P, 512], F16, tag="h")
                    init = 0.0 if warm else carry[:, g:g + 1]
                    nc.vector.tensor_tensor_scan(
                        out=h[:, :cw], data0=a[:, :cw], data1=b[:, :cw],
                        initial=init, op0=OP.mult, op1=OP.add)
                    if c < len(chunks) - 1:
                        nc.gpsimd.tensor_copy(carry[:, g:g + 1], h[:, cw - 1:cw])
                    nc.gpsimd.tensor_tensor(xT[:, g, csl], xT[:, g, csl],
                                            h[:, :cw], op=OP.add)
                    for _, (og, _fin) in pend:
                        og(g)
                if i < NB - 1:
                    for j, (_og, fin) in pend:
                        next_xns[j] = fin(f"n{i}x{j}")
                elif c >= 1:
                    # own piece is final: norm + AllGather it immediately
                    # (piece 0 fires while the last chunk still computes)
                    _, (_og, fin) = pend[0]
                    xf = fin(f"xf{c - 1}")
                    nc.sync.dma_start(cc_in[c - 1], xf[:])
                    nc.gpsimd.collective_compute(
                        kind="AllGather", op=OP.bypass,
                        replica_groups=[list(range(NCORES))],
                        ins=[cc_in[c - 1]], outs=[cc_out[c - 1]])

        # ---- phase C: regather ----
        xg_all = xpool.tile([P, NCORES, NPIECE, KD, PIECE], F16)
        for p in range(NPIECE):
            for c in range(NCORES):
                nc.gpsimd.dma_start(xg_all[:, c, p], cc_out[p][c])

    # ---- readout (own PSUM pool: 8 single-bank tiles) ----
    # token tiles in piece-arrival order: all cores' piece 0, then piece 1
    tiles = [(c, p, hf) for p in range(NPIECE) for c in range(NCORES)
             for hf in range(2)]
    with tc.tile_pool(name="psro", bufs=8, space="PSUM") as rpool:
        for ti, (c, p, hf) in enumerate(tiles):
            tm = c * 4 + p * 2 + hf
            ob = opool.tile([P, VSH], F16, tag="ob")
            for vbg in range(2):
                pss = [rpool.tile([P, VB], F32, tag="pso", name=f"pso{j}")
                       for j in range(4)]
                for kd in range(KD):
                    for j in range(4):
                        vb = vbg * 4 + j
                        nc.tensor.matmul(
                            pss[j][:],
                            lhsT=xg_all[:, c, p, kd, hf * P:(hf + 1) * P],
                            rhs=wro_sb[:, kd, vb * VB:(vb + 1) * VB],
                            start=(kd == 0), stop=(kd == KD - 1))
                for j in range(4):
                    vb = vbg * 4 + j
                    osl = ob[:, vb * VB:(vb + 1) * VB]
                    if j % 2 == 0:
                        nc.scalar.activation(osl, pss[j][:], AF.Copy)
                    else:
                        nc.vector.tensor_copy(osl, pss[j][:])
            dma_eng = nc.sync if ti % 2 == 0 else nc.scalar
            dma_eng.dma_start(out[tm * P:(tm + 1) * P, :], ob[:])


def _get_nc(reps=1):
    key = ("nc", reps)
    if key not in _CACHE:
        _CACHE[key] = _build(reps)
    return _CACHE[key]


def _make_in_maps(input_ids, token_embed, w_hg, norm_scales, final_scale,
                  readout_weight):
    ids = np.asarray(input_ids).reshape(-1).astype(np.int64)
    emb = np.ascontiguousarray(np.asarray(token_embed, np.float32)
                               ).astype(np.float16)
    wq = np.ascontiguousarray(
        np.asarray(norm_scales, np.float32)[:, :, None]
        * np.asarray(w_hg, np.float32)).astype(np.float16)
    wro_full = (np.asarray(final_scale, np.float32)[:, None]
                * np.asarray(readout_weight, np.float32))
    ones = np.ones((P, P), np.float16)
    identm = np.eye(P, dtype=np.float16)
    epsc = np.full((P, 1), EPS, np.float32)
    in_maps = []
    for core in range(NCORES):
        start = core * CHUNK
        widx = (np.arange(start - W, start + CHUNK) % SEQ).astype(np.int64)
        # partition-major layout: idx[p, g] = window token g*128 + p
        idxm = np.ascontiguousarray(
            ids[widx].astype(np.int32).reshape(TW // P, P).T)
        mask = np.ones((P, WW), np.float16)
        if core == 0:
            mask[:] = 0.0
        wro = np.ascontiguousarray(
            wro_full[:, core * VSH:(core + 1) * VSH]).astype(np.float16)
        in_maps.append(dict(emb=emb, idx=idxm, wq=wq, wro=wro, ones_in=ones,
                            ident_in=identm, eps_in=epsc, bmask=mask))
    return in_maps


def kernel(input_ids, token_embed, w_hg, norm_scales, final_scale,
           readout_weight):
    nc = _get_nc()
    in_maps = _make_in_maps(input_ids, token_embed, w_hg, norm_scales,
                            final_scale, readout_weight)
    res = run_bass_kernel_spmd(nc, in_maps, core_ids=list(range(NCORES)))
    logits = np.concatenate(
        [np.asarray(res.results[c]["out"]).astype(np.float32)
         for c in range(NCORES)], axis=1)
    return logits.reshape(1, SEQ, V)


# revision 21
# speedup vs baseline: 1.0511x; 1.0315x over previous
"""Trainium2 Bass kernel for nn_CoreAmplifierLM (minGRU LM, 4 blocks).

Strategy (8 NeuronCores, SPMD):
  - Sequence-parallel blocks: core k owns tokens [k*512, (k+1)*512) plus a
    redundant W=128-token warmup prefix (recurrence decay prod(a) <= 1.5e-3
    over 128 tokens makes the h=0 warm start exact to ~1e-4 in the logits).
    Core 0's warmup wraps the sequence end; a b-mask zeroes its scan input.
  - fp16 residual stream: the embedding table is host-cast to fp16, x^T
    lives as [P, KD, TW] fp16 (D on partitions). Each block processes one
    128-token warmup chunk and ONE merged 512-token main chunk (a [128,512]
    fp32 PSUM tile is exactly one bank and 512 is the max moving-operand
    free size), minimizing per-op overhead and engine-queue boundaries.
    g(h) uses the exact identity relu(h)+sigmoid(min(h,0)) = max(h+0.5,
    sigmoid(h)). Work is spread over scalar (sigmoids, sqrt), vector
    (norms, g(), b, scans) and gpsimd (1-z, residual adds, carries).
    The next block's norms are emitted inside the current block so chunk
    matmuls never wait on the norm chain.
  - Readout is vocab-sharded (tensor parallel): each core holds an fp16
    [512, 4000] readout slice in SBUF (preloaded at t=0) and computes
    logits[:, vslice] for all 4096 tokens. The final-normed x is exchanged
    with TWO pipelined 256-token AllGathers (the first fires mid-block-4;
    the last block keeps split 256-token chunks for this), and the readout
    consumes token tiles in piece-arrival order so the exchange mostly
    hides under matmul work. Logits are assembled per 128-token tile into
    a [128, 4000] fp16 staging tile and written with one contiguous 1 MB
    DMA, widened on host.
"""
import numpy as np
from contextlib import ExitStack

import concourse.bass as bass
import concourse.mybir as mybir
import concourse.tile as tile
from concourse import bacc
from concourse.bass_utils import run_bass_kernel_spmd

P = 128
D = 512
V = 32000
SEQ = 4096
NB = 4
NCORES = 8
CHUNK = SEQ // NCORES          # 512 own tokens per core
W = 128                        # warmup window (gather-layout granularity)
WS, WW = 64, 64                # effective warm scan: tokens [64, 128)
TW = CHUNK + W                 # 640-token window per core
PIECE = 256                    # AllGather piece (tokens)
NPIECE = CHUNK // PIECE        # 2 pieces per core
KD = D // P                    # 4 contraction chunks
MH = 2 * D // P                # 8 output-channel chunks of hg
NG = KD                        # 4 channel groups (hidden dim)
VSH = V // NCORES              # 4000 vocab per core
VB = 500                       # vocab block for readout (one PSUM bank)
NVB = VSH // VB                # 8
TM = SEQ // P                  # 32 token m-chunks in readout
EPS = 1e-6

# per-block token chunks (start, width): one merged 512-token main chunk,
# except the last block which splits it so the first AllGather fires early
CHUNKS_MAIN = [(WS, WW), (W, 512)]
CHUNKS_LAST = [(WS, WW), (W, 256), (W + 256, 256)]

F32 = mybir.dt.float32
F16 = mybir.dt.float16
I32 = mybir.dt.int32
AF = mybir.ActivationFunctionType
OP = mybir.AluOpType

_CACHE = {}


def _build(reps=1):
    nc = bacc.Bacc("TRN2", target_bir_lowering=False, debug=False,
                   enable_asserts=True, num_devices=NCORES)

    emb = nc.dram_tensor("emb", [V, D], F16, kind="ExternalInput").ap()
    idx = nc.dram_tensor("idx", [P, TW // P], I32, kind="ExternalInput").ap()
    wq = nc.dram_tensor("wq", [NB, D, 2 * D], F16, kind="ExternalInput").ap()
    wro = nc.dram_tensor("wro", [D, VSH], F16, kind="ExternalInput").ap()
    ones_in = nc.dram_tensor("ones_in", [P, P], F16, kind="ExternalInput").ap()
    ident_in = nc.dram_tensor("ident_in", [P, P], F16,
                              kind="ExternalInput").ap()
    eps_in = nc.dram_tensor("eps_in", [P, 1], F32, kind="ExternalInput").ap()
    bmask = nc.dram_tensor("bmask", [P, WW], F16, kind="ExternalInput").ap()
    out = nc.dram_tensor("out", [SEQ, VSH], F16, kind="ExternalOutput").ap()
    cc_in = [nc.dram_tensor(f"cc_in{p}", [P, KD, PIECE], F16,
                            kind="Internal").ap() for p in range(NPIECE)]
    cc_out = [nc.dram_tensor(f"cc_out{p}", [NCORES, P, KD, PIECE], F16,
                             kind="Internal", addr_space="Shared").ap()
              for p in range(NPIECE)]

    with tile.TileContext(nc) as tc, ExitStack() as ctx:
        cpool = ctx.enter_context(tc.tile_pool(name="const", bufs=1))
        xpool = ctx.enter_context(tc.tile_pool(name="xT", bufs=1))
        gpool = ctx.enter_context(tc.tile_pool(name="gather", bufs=3))
        wpool = ctx.enter_context(tc.tile_pool(name="w", bufs=2))
        tpool = ctx.enter_context(tc.tile_pool(name="normtmp", bufs=1))
        epool = ctx.enter_context(tc.tile_pool(name="elem", bufs=3))
        hpool = ctx.enter_context(tc.tile_pool(name="h", bufs=3))
        opool = ctx.enter_context(tc.tile_pool(name="obuf", bufs=3))

        # tiny front-loaded constants (sync queue) so the gather can start
        idx_sb = cpool.tile([P, TW // P], I32)
        nc.sync.dma_start(idx_sb[:], idx)
        mask_sb = cpool.tile([P, WW], F16)
        nc.sync.dma_start(mask_sb[:], bmask)
        ones_h = cpool.tile([P, P], F16)
        nc.sync.dma_start(ones_h[:], ones_in)
        ident = cpool.tile([P, P], F16)
        nc.sync.dma_start(ident[:], ident_in)
        carry = cpool.tile([P, NG], F16)
        eps_sb = cpool.tile([P, 1], F32)
        nc.sync.dma_start(eps_sb[:], eps_in)
        # big preload: the fp16 readout slice rides the sync queue behind the
        # tiny constants (it is only needed at readout time, ~150us in)
        wro_sb = cpool.tile([P, KD, VSH], F16)
        nc.sync.dma_start(wro_sb[:], wro.rearrange("(kd p) v -> p kd v", p=P))

        args = (nc, tc, ctx, cpool, xpool, gpool, wpool, tpool, epool, hpool,
                opool, ident, ones_h, mask_sb, idx_sb, carry,
                eps_sb, wro_sb, emb, wq, out, cc_in, cc_out)
        if reps == 1:
            _body(*args)
        else:
            with tc.For_i(0, reps, 1):
                _body(*args)

    nc.compile()
    return nc


def _body(nc, tc, ctx, cpool, xpool, gpool, wpool, tpool, epool, hpool,
          opool, ident, ones_h, mask_sb, idx_sb, carry,
          eps_sb, wro_sb, emb, wq, out, cc_in, cc_out):
    psbox = [None]
    with tc.tile_pool(name="psb", bufs=1, space="PSUM") as pspool:
        psbox[0] = (pspool, "ps", 7)
        # ---- phase A: gather embedding rows, transpose to xT [P, KD, TW] ----
        xT = xpool.tile([P, KD, TW], F16)

        def gather_group(g):
            xr = gpool.tile([P, D], F16, tag="xr")
            nc.gpsimd.indirect_dma_start(
                out=xr[:], out_offset=None, in_=emb,
                in_offset=bass.IndirectOffsetOnAxis(ap=idx_sb[:, g:g + 1], axis=0))
            for d in range(KD):
                ps_t = psbox[0][0].tile([P, 2 * 512], F16, tag="pst", bufs=1,
                                        name="ps_t")
                nc.tensor.transpose(ps_t[:, :P], xr[:, d * P:(d + 1) * P],
                                    ident[:])
                nc.vector.tensor_copy(xT[:, d, g * P:(g + 1) * P], ps_t[:, :P])

        def norm_begin(clo, width, tag):
            """Incremental rms-norm: on_group(g) squares + accumulates group
            g's partition-sum right after that group's residual add, so the
            norm finishes ~one op after the chunk's last add."""
            csl = slice(clo, clo + width)
            pool, ptag, pbufs = psbox[0]
            ps_n = pool.tile([P, 512], F32, tag=ptag, bufs=pbufs,
                             name=f"psn_{tag}")

            def on_group(g):
                x2g = tpool.tile([P, width], F16, tag=f"x2g{width}", bufs=2,
                                 name=f"x2g_{tag}_{g}")
                nc.vector.tensor_tensor(x2g[:], xT[:, g, csl], xT[:, g, csl],
                                        op=OP.mult)
                nc.tensor.matmul(ps_n[:, :width], lhsT=ones_h[:], rhs=x2g[:],
                                 start=(g == 0), stop=(g == NG - 1))

            def finish(xn_tag):
                # rstd = 1/sqrt(mean + eps), one table op (input is >= 0)
                rstd_h = tpool.tile([P, width], F16, tag=f"rstdh{width}",
                                    name=f"rs_{tag}")
                nc.scalar.activation(rstd_h[:], ps_n[:, :width],
                                     AF.Abs_reciprocal_sqrt,
                                     scale=1.0 / D, bias=eps_sb[:, :1])
                xn = tpool.tile([P, KD, width], F16, tag=xn_tag,
                                name=f"xn_{tag}")
                nc.vector.tensor_tensor(
                    xn[:], xT[:, :, csl],
                    rstd_h[:, None, :].to_broadcast([P, KD, width]),
                    op=OP.mult)
                return xn

            return on_group, finish

        def rms_xn(clo, width, xn_tag="xn"):
            og, fin = norm_begin(clo, width, xn_tag)
            for g in range(NG):
                og(g)
            return fin(xn_tag)

        # the first norm only needs gather group 0; hoisting it between the
        # gathers lets block 0 start earlier
        gather_group(0)
        xn00 = rms_xn(WS, WW, xn_tag="xn0")
        for g in range(1, TW // P):
            gather_group(g)

        # ---- phase B: minGRU blocks ----
        next_xns = None
        for i in range(NB):
            chunks = CHUNKS_LAST if i == NB - 1 else CHUNKS_MAIN
            nchunks = CHUNKS_LAST if i == NB - 2 else CHUNKS_MAIN
            w_sb = wpool.tile([P, KD, MH, P], F16, tag="w")
            nc.scalar.dma_start(
                w_sb[:], wq[i].rearrange("(kd p) (mh j) -> p kd mh j", p=P, j=P))
            if i == 0:
                xns = [xn00] + [rms_xn(clo, cw, xn_tag=f"b0xn{c}")
                                for c, (clo, cw) in enumerate(chunks) if c > 0]
            else:
                xns = next_xns
            next_xns = [None] * len(nchunks)
            for c, (clo, cw) in enumerate(chunks):
                csl = slice(clo, clo + cw)
                warm = c == 0
                xn = xns[c]
                # pending norms over this chunk's tokens, finished right
                # after the last group's residual add lands
                pend = []
                if i < NB - 1:
                    if warm:
                        pend = [(0, norm_begin(WS, WW, f"n{i}c0"))]
                    else:
                        pend = [(j, norm_begin(nlo, nw, f"n{i}c{j}"))
                                for j, (nlo, nw) in enumerate(nchunks)
                                if j > 0]
                elif c >= 1:
                    pend = [(c, norm_begin(clo, cw, f"pc{c}"))]
                for g in range(NG):
                    ps_h = pspool.tile([P, 512], F32, tag="ps", bufs=7)
                    ps_g = pspool.tile([P, 512], F32, tag="ps", bufs=7)
                    for kd in range(KD):
                        nc.tensor.matmul(ps_h[:, :cw], lhsT=w_sb[:, kd, g, :],
                                         rhs=xn[:, kd, :],
                                         start=(kd == 0), stop=(kd == KD - 1))
                    for kd in range(KD):
                        nc.tensor.matmul(ps_g[:, :cw], lhsT=w_sb[:, kd, g + NG, :],
                                         rhs=xn[:, kd, :],
                                         start=(kd == 0), stop=(kd == KD - 1))
                    z = epool.tile([P, 512], F16, tag="z")
                    nc.scalar.activation(z[:, :cw], ps_g[:, :cw], AF.Sigmoid)
                    # a = 1 - z = sigmoid(-gate), off the scalar engine
                    a = epool.tile([P, 512], F16, tag="a")
                    nc.gpsimd.tensor_scalar(a[:, :cw], z[:, :cw], -1.0, 1.0,
                                            op0=OP.mult, op1=OP.add)
                    sh = epool.tile([P, 512], F16, tag="sh")
                    nc.scalar.activation(sh[:, :cw], ps_h[:, :cw], AF.Sigmoid)
                    # g(h) = relu(h) + sigmoid(min(h, 0)) = max(h + 0.5,
                    # sigmoid(h)) exactly (sigmoid is below h+0.5 iff h > 0)
                    gg = epool.tile([P, 512], F16, tag="gg")
                    nc.vector.scalar_tensor_tensor(
                        out=gg[:, :cw], in0=ps_h[:, :cw], scalar=0.5,
                        in1=sh[:, :cw], op0=OP.add, op1=OP.max)
                    if warm:
                        # zero warmup b on core 0 (mask = 0 there, 1 elsewhere)
                        nc.vector.tensor_tensor(z[:, :cw], z[:, :cw],
                                                mask_sb[:], op=OP.mult)
                    b = epool.tile([P, 512], F16, tag="b")
                    nc.vector.tensor_tensor(b[:, :cw], z[:, :cw], gg[:, :cw],
                                            op=OP.mult)
                    h = hpool.tile([P, 512], F16, tag="h")
                    init = 0.0 if warm else carry[:, g:g + 1]
                    nc.vector.tensor_tensor_scan(
                        out=h[:, :cw], data0=a[:, :cw], data1=b[:, :cw],
                        initial=init, op0=OP.mult, op1=OP.add)
                    if c < len(chunks) - 1:
                        nc.gpsimd.tensor_copy(carry[:, g:g + 1], h[:, cw - 1:cw])
                    nc.gpsimd.tensor_tensor(xT[:, g, csl], xT[:, g, csl],
                                            h[:, :cw], op=OP.add)
                    for _, (og, _fin) in pend:
                        og(g)
                if i < NB - 1:
                    for j, (_og, fin) in pend:
                        next_xns[j] = fin(f"n{i}x{j}")
                elif c >= 1:
                    # own piece is final: norm + AllGather it immediately
                    # (piece 0 fires while the last chunk still computes)
                    _, (_og, fin) = pend[0]
                    xf = fin(f"xf{c - 1}")
                    nc.sync.dma_start(cc_in[c - 1], xf[:])
                    nc.gpsimd.collective_compute(
                        kind="AllGather", op=OP.bypass,
                        replica_groups=[list(range(NCORES))],
                        ins=[cc_in[c - 1]], outs=[cc_out[c - 1]])

        # ---- phase C: regather ----
        xg_all = xpool.tile([P, NCORES, NPIECE, KD, PIECE], F16)
        for p in range(NPIECE):
            for c in range(NCORES):
                nc.gpsimd.dma_start(xg_all[:, c, p], cc_out[p][c])

    # ---- readout (own PSUM pool: 8 single-bank tiles) ----
    # token tiles in piece-arrival order: all cores' piece 0, then piece 1
    tiles = [(c, p, hf) for p in range(NPIECE) for c in range(NCORES)
             for hf in range(2)]
    with tc.tile_pool(name="psro", bufs=8, space="PSUM") as rpool:
        for ti, (c, p, hf) in enumerate(tiles):
            tm = c * 4 + p * 2 + hf
            ob = opool.tile([P, VSH], F16, tag="ob")
            for vbg in range(2):
                pss = [rpool.tile([P, VB], F32, tag="pso", name=f"pso{j}")
                       for j in range(4)]
                for kd in range(KD):
                    for j in range(4):
                        vb = vbg * 4 + j
                        nc.tensor.matmul(
                            pss[j][:],
                            lhsT=xg_all[:, c, p, kd, hf * P:(hf + 1) * P],
                            rhs=wro_sb[:, kd, vb * VB:(vb + 1) * VB],
                            start=(kd == 0), stop=(kd == KD - 1))
                for j in range(4):
                    vb = vbg * 4 + j
                    osl = ob[:, vb * VB:(vb + 1) * VB]
                    if j % 2 == 0:
                        nc.scalar.activation(osl, pss[j][:], AF.Copy)
                    else:
                        nc.vector.tensor_copy(osl, pss[j][:])
                # half-tile store right after this vbg's copies: the tail
                # drain is one half-write instead of a full-tile write
                dma_eng = nc.sync if (2 * ti + vbg) % 2 == 0 else nc.scalar
                half = slice(vbg * 4 * VB, (vbg + 1) * 4 * VB)
                dma_eng.dma_start(out[tm * P:(tm + 1) * P, half],
                                  ob[:, half])


def _get_nc(reps=1):
    key = ("nc", reps)
    if key not in _CACHE:
        _CACHE[key] = _build(reps)
    return _CACHE[key]


def _make_in_maps(input_ids, token_embed, w_hg, norm_scales, final_scale,
                  readout_weight):
    ids = np.asarray(input_ids).reshape(-1).astype(np.int64)
    emb = np.ascontiguousarray(np.asarray(token_embed, np.float32)
                               ).astype(np.float16)
    wq = np.ascontiguousarray(
        np.asarray(norm_scales, np.float32)[:, :, None]
        * np.asarray(w_hg, np.float32)).astype(np.float16)
    wro_full = (np.asarray(final_scale, np.float32)[:, None]
                * np.asarray(readout_weight, np.float32))
    ones = np.ones((P, P), np.float16)
    identm = np.eye(P, dtype=np.float16)
    epsc = np.full((P, 1), EPS, np.float32)
    in_maps = []
    for core in range(NCORES):
        start = core * CHUNK
        widx = (np.arange(start - W, start + CHUNK) % SEQ).astype(np.int64)
        # partition-major layout: idx[p, g] = window token g*128 + p
        idxm = np.ascontiguousarray(
            ids[widx].astype(np.int32).reshape(TW // P, P).T)
        mask = np.ones((P, WW), np.float16)
        if core == 0:
            mask[:] = 0.0
        wro = np.ascontiguousarray(
            wro_full[:, core * VSH:(core + 1) * VSH]).astype(np.float16)
        in_maps.append(dict(emb=emb, idx=idxm, wq=wq, wro=wro, ones_in=ones,
                            ident_in=identm, eps_in=epsc, bmask=mask))
    return in_maps


def kernel(input_ids, token_embed, w_hg, norm_scales, final_scale,
           readout_weight):
    nc = _get_nc()
    in_maps = _make_in_maps(input_ids, token_embed, w_hg, norm_scales,
                            final_scale, readout_weight)
    res = run_bass_kernel_spmd(nc, in_maps, core_ids=list(range(NCORES)))
    logits = np.concatenate(
        [np.asarray(res.results[c]["out"]).astype(np.float32)
         for c in range(NCORES)], axis=1)
    return logits.reshape(1, SEQ, V)
